# revision 2
# baseline (speedup 1.0000x reference)
"""Cumulative link (ordinal) loss on 8 Trainium2 NeuronCores.

loss = mean_i [ -ln( sigmoid(hi_i - x_i) - sigmoid(lo_i - x_i) + eps ) ]
with per-label thresholds hi = [0,1,2,3,+inf][l], lo = [-inf,0,1,2,3][l].

Branch-free device formulation (l = label as float, G = l - x):
    S1 = sigmoid(G)          # = sigmoid(hi - x) when l <= 3
    S2 = sigmoid(G - 1)      # = sigmoid(lo - x) when l >= 1
    A  = max(S1, l - 3)      # l==4  ->  1      (l-3 = 1 > S1), else S1
    B  = min(S2, l)          # l==0  ->  0      (l = 0 < S2),  else S2
    p  = A - B
    partial[p] = sum_free ln(p + eps)      (ACT Ln with accum_out)
Host: loss = -sum(partials) / B.

Sharding: pure data parallel, 1/8 of the batch per core, laid out
[128 partitions x 8192 free].  Labels are read densely as int32 pairs
(the int64 low/high words); the low word feeds compute via a stride-2
access pattern, so no int64 support is needed on-device.
"""

import numpy as np

B_TOTAL = 8388608
N_CORES = 8
P = 128
SHARD = B_TOTAL // N_CORES          # 1048576 per core
M = SHARD // P                      # 8192 free-dim columns per core
T = 2048                            # tile width (columns)
NT = M // T
EPS = 1e-8

# engine for the B = min(S2, l) op: "vector" or "gpsimd"
# (gpsimd rejected: TensorTensor opcode not legal on Pool engine on TRN2)
B_ENGINE = "vector"

_NC = None


def _build_nc():
    import concourse.bacc as bacc
    import concourse.mybir as mybir
    from concourse import tile

    f32 = mybir.dt.float32
    i32 = mybir.dt.int32
    bf16 = mybir.dt.bfloat16
    Alu = mybir.AluOpType
    Act = mybir.ActivationFunctionType

    nc = bacc.Bacc("TRN2", target_bir_lowering=False, debug=False)

    x_dram = nc.dram_tensor("logits", (P, M), f32, kind="ExternalInput")
    l_dram = nc.dram_tensor("labels", (P, 2 * M), i32, kind="ExternalInput")
    o_dram = nc.dram_tensor("out", (P, 1), f32, kind="ExternalOutput")

    with tile.TileContext(nc) as tc:
        with (
            tc.tile_pool(name="io", bufs=2) as iop,
            tc.tile_pool(name="work", bufs=2) as wp,
            tc.tile_pool(name="persist", bufs=1) as pp,
        ):
            bias_m1 = pp.tile([P, 1], f32, tag="bias_m1")
            nc.vector.memset(bias_m1[:], -1.0)
            bias_eps = pp.tile([P, 1], f32, tag="bias_eps")
            nc.vector.memset(bias_eps[:], EPS)

            p_full = pp.tile([P, M], f32, tag="p_full")
            acc = pp.tile([P, 1], f32, tag="acc")

            for t in range(NT):
                xt = iop.tile([P, T], f32, tag="x")
                lt = iop.tile([P, T, 2], i32, tag="l")
                nc.sync.dma_start(out=xt[:], in_=x_dram[:, t * T:(t + 1) * T])
                nc.sync.dma_start(
                    out=lt[:], in_=l_dram[:, t * 2 * T:(t + 1) * 2 * T]
                )
                lev = lt[:, :, 0]          # int32 low words, stride 2

                g = wp.tile([P, T], f32, tag="g")
                nc.vector.tensor_tensor(
                    out=g[:], in0=lev, in1=xt[:], op=Alu.subtract
                )
                s1 = wp.tile([P, T], f32, tag="s1")
                nc.scalar.activation(s1[:], g[:], Act.Sigmoid)
                s2 = wp.tile([P, T], f32, tag="s2")
                nc.scalar.activation(s2[:], g[:], Act.Sigmoid, bias=bias_m1[:])

                a = wp.tile([P, T], f32, tag="a")
                nc.vector.scalar_tensor_tensor(
                    out=a[:], in0=lev, scalar=3.0, in1=s1[:],
                    op0=Alu.subtract, op1=Alu.max,
                )
                b = wp.tile([P, T], f32, tag="b")
                b_eng = nc.gpsimd if B_ENGINE == "gpsimd" else nc.vector
                b_eng.tensor_tensor(out=b[:], in0=lev, in1=s2[:], op=Alu.min)

                nc.vector.tensor_tensor(
                    out=p_full[:, t * T:(t + 1) * T],
                    in0=a[:], in1=b[:], op=Alu.subtract,
                )

            # one table switch sigmoid -> ln; Ln writes in place over p_full,
            # accum_out gives the per-partition sum of ln(p + eps)
            nc.scalar.activation(
                p_full[:], p_full[:], Act.Ln, bias=bias_eps[:], accum_out=acc[:]
            )
            nc.sync.dma_start(out=o_dram[:], in_=acc[:])

    nc.compile()
    return nc


def get_nc():
    global _NC
    if _NC is None:
        _NC = _build_nc()
    return _NC


def make_in_maps(logits, labels):
    x = np.ascontiguousarray(np.asarray(logits, dtype=np.float32)).reshape(B_TOTAL)
    lab = np.asarray(labels)
    if lab.dtype != np.int64:
        lab = lab.astype(np.int64)
    lab = np.ascontiguousarray(lab).reshape(B_TOTAL)
    lab32 = lab.view(np.int32)          # (2*B,) interleaved low/high words
    in_maps = []
    for c in range(N_CORES):
        xs = x[c * SHARD:(c + 1) * SHARD].reshape(P, M)
        ls = lab32[c * 2 * SHARD:(c + 1) * 2 * SHARD].reshape(P, 2 * M)
        in_maps.append({"logits": xs, "labels": ls})
    return in_maps


def run(logits, labels, trace=False):
    """Returns (loss_scalar_f32, BassKernelResults)."""
    from concourse.bass_utils import run_bass_kernel_spmd

    nc = get_nc()
    in_maps = make_in_maps(logits, labels)
    res = run_bass_kernel_spmd(
        nc, in_maps, core_ids=list(range(N_CORES)), trace=trace
    )
    total = 0.0
    for r in res.results:
        total += r["out"].astype(np.float64).sum()
    loss = np.float32(-total / B_TOTAL)
    return np.asarray(loss), res


def kernel(logits, labels):
    out, _ = run(logits, labels, trace=False)
    return out
